# revision 2
# baseline (speedup 1.0000x reference)
"""VQ codebook (cosine-sim top-32 gather-sum) Trainium2 kernel.

Strategy (data-parallel over batch, 8 cores, codebook replicated):
  per core: x shard [512, 1024], codebook [16384, 1024]
  phase A: stream codebook in 512-row chunks; normalize rows; hi/lo bf16
    split; xbar-transpose to [d, n] layout; 3-pass bf16 matmul
    (xh.ch + xh.cl + xl.ch) accumulated in PSUM = fp32-accurate cosine sims;
    per-chunk top-8 candidates (max8 + max_index).
  phase B: merge 256 candidates/row -> exact top-32 indices via threshold +
    index-compaction; 32 accumulating indirect-DMA gathers of fp32 codebook
    rows -> x_hat.
"""
import numpy as np

P = 128
D = 1024
B_LOCAL = 512
NCB = 16384
CHUNK = 512
NCH = NCB // CHUNK  # 32
K = 32
NB = B_LOCAL // P  # 4 b-tiles
G = D // P  # 8 k-tiles
N_CORES = 8

_ctr = [0]


def _split_multi_waits(nc, mybir, limit=1):
    """This walrus build rejects >1 sync wait on Drain/Branch/DmaTransposeAnt
    (and the exit drains Tile emits routinely carry 2+). Hoist extras into
    standalone EventSemaphore instructions."""
    for fn in nc.m.functions:
        for bb in fn.blocks:
            insts = bb.instructions
            i = 0
            while i < len(insts):
                inst = insts[i]
                si = inst.sync_info
                if si is not None and len(si.on_wait) > limit:
                    waits = list(si.on_wait)
                    extra, keep = waits[:-limit], waits[-limit:]
                    new_insts = []
                    for w in extra:
                        _ctr[0] += 1
                        ev = mybir.InstEventSemaphore(
                            name=f"waitfix-{_ctr[0]}", ins=[], outs=[]
                        )
                        ev.engine = inst.engine
                        ev.sync_info = mybir.SyncInfo(on_wait=[w], on_update=[])
                        new_insts.append(ev)
                    inst.sync_info = mybir.SyncInfo(
                        on_wait=keep, on_update=list(si.on_update)
                    )
                    insts[i:i] = new_insts
                    i += len(new_insts)
                i += 1


def build_kernel():
    import concourse.bass as bass
    import concourse.mybir as mybir
    import concourse.tile as tile

    f32 = mybir.dt.float32
    bf16 = mybir.dt.bfloat16
    i32 = mybir.dt.int32
    u32 = mybir.dt.uint32
    Alu = mybir.AluOpType
    Act = mybir.ActivationFunctionType

    nc = bass.Bass("TRN2", target_bir_lowering=False, debug=False)
    x = nc.dram_tensor("x", [B_LOCAL, D], f32, kind="ExternalInput")
    cb = nc.dram_tensor("codebook", [NCB, D], f32, kind="ExternalInput")
    xhat = nc.dram_tensor("x_hat", [B_LOCAL, D], f32, kind="ExternalOutput")

    with tile.TileContext(nc) as tc:
        with (
            tc.tile_pool(name="persist", bufs=1) as pp,
            tc.tile_pool(name="stream", bufs=2) as sp,
            tc.tile_pool(name="psum", bufs=2, space="PSUM") as ps,
        ):
            # ---------------- x prep ----------------
            x_s = pp.tile([P, NB, D], f32)
            nc.sync.dma_start(x_s[:], x.ap().rearrange("(t p) d -> p t d", p=P))
            sq_junk = pp.tile([P, D], bf16)
            xn2 = pp.tile([P, NB], f32)
            for t in range(NB):
                nc.scalar.activation(
                    sq_junk[:], x_s[:, t, :], Act.Square,
                    accum_out=xn2[:, t : t + 1],
                )
            xr2 = pp.tile([P, NB], f32)
            nc.vector.tensor_scalar_max(xr2[:], xn2[:], 1e-16)
            nc.vector.reciprocal(xr2[:], xr2[:])
            xr = pp.tile([P, NB], f32)
            nc.scalar.activation(xr[:], xr2[:], Act.Sqrt)
            xh = pp.tile([P, NB, D], bf16)
            xl = pp.tile([P, NB, D], bf16)
            for t in range(NB):
                nc.vector.tensor_scalar_mul(
                    x_s[:, t, :], x_s[:, t, :], xr[:, t : t + 1]
                )
                nc.vector.tensor_copy(xh[:, t, :], x_s[:, t, :])
                nc.vector.tensor_tensor(
                    out=xl[:, t, :], in0=x_s[:, t, :], in1=xh[:, t, :],
                    op=Alu.subtract,
                )
            xhT = pp.tile([P, G, B_LOCAL], bf16)
            xlT = pp.tile([P, G, B_LOCAL], bf16)
            for t in range(NB):
                nc.sync.dma_start_transpose(
                    xhT[:, :, t * P : (t + 1) * P], xh[:, t, :]
                )
                nc.sync.dma_start_transpose(
                    xlT[:, :, t * P : (t + 1) * P], xl[:, t, :]
                )

            # candidate arrays (persist across chunk loop)
            cand_v = [pp.tile([P, NCH * 8], f32, tag=f"cv{m}", name=f"cv{m}") for m in range(NB)]
            cand_i = [pp.tile([P, NCH * 8], f32, tag=f"ci{m}", name=f"ci{m}") for m in range(NB)]

            # ---------------- codebook streaming ----------------
            for j in range(NCH):
                c_s = sp.tile([P, NB, D], f32, tag="c_s")
                nc.sync.dma_start(
                    c_s[:],
                    cb.ap()[j * CHUNK : (j + 1) * CHUNK, :].rearrange(
                        "(t p) d -> p t d", p=P
                    ),
                )
                nrm2 = sp.tile([P, NB], f32, tag="nrm2")
                cjunk = sp.tile([P, D], bf16, tag="cjunk")
                for t in range(NB):
                    nc.scalar.activation(
                        cjunk[:], c_s[:, t, :], Act.Square,
                        accum_out=nrm2[:, t : t + 1],
                    )
                nc.vector.tensor_scalar_max(nrm2[:], nrm2[:], 1e-16)
                nc.vector.reciprocal(nrm2[:], nrm2[:])
                rcp = sp.tile([P, NB], f32, tag="rcp")
                nc.scalar.activation(rcp[:], nrm2[:], Act.Sqrt)

                ch = sp.tile([P, NB, D], bf16, tag="ch")
                cl = sp.tile([P, NB, D], bf16, tag="cl")
                for t in range(NB):
                    nc.vector.tensor_scalar_mul(
                        c_s[:, t, :], c_s[:, t, :], rcp[:, t : t + 1]
                    )
                    nc.vector.tensor_copy(ch[:, t, :], c_s[:, t, :])
                    nc.vector.tensor_tensor(
                        out=cl[:, t, :], in0=c_s[:, t, :], in1=ch[:, t, :],
                        op=Alu.subtract,
                    )
                chT = sp.tile([P, G, CHUNK], bf16, tag="chT")
                clT = sp.tile([P, G, CHUNK], bf16, tag="clT")
                for t in range(NB):
                    nc.sync.dma_start_transpose(
                        chT[:, :, t * P : (t + 1) * P], ch[:, t, :]
                    )
                    nc.sync.dma_start_transpose(
                        clT[:, :, t * P : (t + 1) * P], cl[:, t, :]
                    )

                for m in range(NB):
                    pm = ps.tile([P, CHUNK], f32, tag=f"pm{m}", space="PSUM")
                    for g in range(G):
                        nc.tensor.matmul(
                            pm[:], xhT[:, g, m * P : (m + 1) * P], chT[:, g, :],
                            start=(g == 0), stop=False,
                        )
                    for g in range(G):
                        nc.tensor.matmul(
                            pm[:], xhT[:, g, m * P : (m + 1) * P], clT[:, g, :],
                            start=False, stop=False,
                        )
                    for g in range(G):
                        nc.tensor.matmul(
                            pm[:], xlT[:, g, m * P : (m + 1) * P], chT[:, g, :],
                            start=False, stop=(g == G - 1),
                        )
                    sims = sp.tile([P, CHUNK], f32, tag=f"sims{m}")
                    nc.scalar.copy(sims[:], pm[:])
                    cv8 = cand_v[m][:, j * 8 : (j + 1) * 8]
                    nc.vector.max(out=cv8, in_=sims[:])
                    tmpi = sp.tile([P, 8], u32, tag=f"tmpi{m}")
                    nc.vector.max_index(out=tmpi[:], in_max=cv8, in_values=sims[:])
                    nc.vector.tensor_scalar(
                        out=cand_i[m][:, j * 8 : (j + 1) * 8],
                        in0=tmpi[:],
                        scalar1=float(j * CHUNK + 1),
                        scalar2=None,
                        op0=Alu.add,
                    )

            # ---------------- merge + select + gather ----------------
            NC = NCH * 8  # 256 candidates
            for m in range(NB):
                mr_a = sp.tile([P, NC], f32, tag="mr_a")
                mr_b = sp.tile([P, NC], f32, tag="mr_b")
                mx = sp.tile([P, 8], f32, tag="mx")
                src = cand_v[m]
                for r in range(4):
                    nc.vector.max(out=mx[:], in_=src[:])
                    if r < 3:
                        dst = mr_a if r % 2 == 0 else mr_b
                        nc.vector.match_replace(
                            out=dst[:], in_to_replace=mx[:], in_values=src[:],
                            imm_value=-1e30,
                        )
                        src = dst
                # threshold t32 = 32nd value = mx[:, 7] of round 3
                selm = sp.tile([P, NC], f32, tag="selm")
                nc.vector.tensor_scalar(
                    out=selm[:], in0=cand_v[m][:], scalar1=mx[:, 7:8],
                    scalar2=None, op0=Alu.is_ge,
                )
                seli = sp.tile([P, NC], f32, tag="seli")
                nc.vector.tensor_tensor(
                    out=seli[:], in0=selm[:], in1=cand_i[m][:], op=Alu.mult
                )
                win = sp.tile([P, K], f32, tag="win")
                srci = seli
                for r in range(4):
                    w8 = win[:, r * 8 : (r + 1) * 8]
                    nc.vector.max(out=w8, in_=srci[:])
                    if r < 3:
                        dsti = mr_a if r % 2 == 0 else mr_b
                        nc.vector.match_replace(
                            out=dsti[:], in_to_replace=w8, in_values=srci[:],
                            imm_value=0.0,
                        )
                        srci = dsti
                idx32 = sp.tile([P, K], i32, tag="idx32")
                nc.vector.tensor_scalar(
                    out=idx32[:], in0=win[:], scalar1=-1.0, scalar2=None,
                    op0=Alu.add,
                )
                xh_m = sp.tile([P, D], f32, tag="xh_m")
                for q in range(K):
                    nc.gpsimd.indirect_dma_start(
                        out=xh_m[:],
                        out_offset=None,
                        in_=cb[:, :],
                        in_offset=bass.IndirectOffsetOnAxis(
                            ap=idx32[:, q : q + 1], axis=0
                        ),
                        compute_op=(Alu.bypass if q == 0 else Alu.add),
                    )
                nc.sync.dma_start(xhat[m * P : (m + 1) * P, :], xh_m[:])

    _split_multi_waits(nc, mybir)
    return nc


_cached = {}


def _get_kernel():
    if "nc" not in _cached:
        _cached["nc"] = build_kernel()
    return _cached["nc"]


def kernel(x, codebook, k):
    from concourse.bass_utils import run_bass_kernel_spmd

    x = np.ascontiguousarray(np.asarray(x, dtype=np.float32))
    codebook = np.ascontiguousarray(np.asarray(codebook, dtype=np.float32))
    assert int(k) == K, f"kernel hardcodes k={K}, got {k}"
    assert x.shape == (N_CORES * B_LOCAL, D)
    assert codebook.shape == (NCB, D)

    nc = _get_kernel()
    in_maps = [
        {"x": x[i * B_LOCAL : (i + 1) * B_LOCAL], "codebook": codebook}
        for i in range(N_CORES)
    ]
    res = run_bass_kernel_spmd(nc, in_maps, list(range(N_CORES)))
    out = np.concatenate([r["x_hat"] for r in res.results], axis=0)
    return out.astype(np.float32)


if __name__ == "__main__":
    rng = np.random.default_rng(0)
    x = rng.standard_normal((N_CORES * B_LOCAL, D), dtype=np.float32)
    cbk = rng.standard_normal((NCB, D), dtype=np.float32)
    out = kernel(x, cbk, K)
    print("kernel ran, out shape", out.shape)


# revision 3
# speedup vs baseline: 814.9558x; 814.9558x over previous
"""VQ codebook (cosine-sim top-32 gather-sum) Trainium2 kernel.

Strategy (data-parallel over batch, 8 cores, codebook replicated):
  per core: x shard [512, 1024], codebook [16384, 1024]
  phase A: stream codebook in 512-row chunks; normalize rows; hi/lo bf16
    split; xbar-transpose to [d, n] layout; 3-pass bf16 matmul
    (xh.ch + xh.cl + xl.ch) accumulated in PSUM = fp32-accurate cosine sims;
    per-chunk top-8 candidates (max8 + max_index).
  phase B: merge 256 candidates/row -> exact top-32 indices via threshold +
    index-compaction; 32 accumulating indirect-DMA gathers of fp32 codebook
    rows -> x_hat.
"""
import numpy as np

P = 128
D = 1024
B_LOCAL = 512
NCB = 16384
CHUNK = 512
NCH = NCB // CHUNK  # 32
K = 32
NB = B_LOCAL // P  # 4 b-tiles
G = D // P  # 8 k-tiles
N_CORES = 8

_ctr = [0]


def _split_multi_waits(nc, mybir, limit=1):
    """This walrus build rejects >1 sync wait on Drain/Branch/DmaTransposeAnt
    (and the exit drains Tile emits routinely carry 2+). Hoist extras into
    standalone EventSemaphore instructions."""
    for fn in nc.m.functions:
        for bb in fn.blocks:
            insts = bb.instructions
            i = 0
            while i < len(insts):
                inst = insts[i]
                si = inst.sync_info
                if si is not None and len(si.on_wait) > limit:
                    waits = list(si.on_wait)
                    extra, keep = waits[:-limit], waits[-limit:]
                    new_insts = []
                    for w in extra:
                        _ctr[0] += 1
                        ev = mybir.InstEventSemaphore(
                            name=f"waitfix-{_ctr[0]}", ins=[], outs=[]
                        )
                        ev.engine = inst.engine
                        ev.sync_info = mybir.SyncInfo(on_wait=[w], on_update=[])
                        new_insts.append(ev)
                    inst.sync_info = mybir.SyncInfo(
                        on_wait=keep, on_update=list(si.on_update)
                    )
                    insts[i:i] = new_insts
                    i += len(new_insts)
                i += 1


def build_kernel(reps=1):
    import concourse.bass as bass
    import concourse.mybir as mybir
    import concourse.tile as tile

    f32 = mybir.dt.float32
    bf16 = mybir.dt.bfloat16
    i32 = mybir.dt.int32
    u32 = mybir.dt.uint32
    Alu = mybir.AluOpType
    Act = mybir.ActivationFunctionType

    nc = bass.Bass("TRN2", target_bir_lowering=False, debug=False)
    x = nc.dram_tensor("x", [B_LOCAL, D], f32, kind="ExternalInput")
    cb = nc.dram_tensor("codebook", [NCB, D], f32, kind="ExternalInput")
    xhat = nc.dram_tensor("x_hat", [B_LOCAL, D], f32, kind="ExternalOutput")

    with tile.TileContext(nc) as tc:
        with (
            tc.tile_pool(name="persist", bufs=1) as pp,
            tc.tile_pool(name="stream", bufs=2) as sp,
            tc.tile_pool(name="psum", bufs=2, space="PSUM") as ps,
        ):
          for _rep in range(reps):
            # ---------------- x prep ----------------
            x_s = pp.tile([P, NB, D], f32)
            nc.sync.dma_start(x_s[:], x.ap().rearrange("(t p) d -> p t d", p=P))
            sq_junk = pp.tile([P, D], bf16)
            xn2 = pp.tile([P, NB], f32)
            for t in range(NB):
                nc.scalar.activation(
                    sq_junk[:], x_s[:, t, :], Act.Square,
                    accum_out=xn2[:, t : t + 1],
                )
            xr2 = pp.tile([P, NB], f32)
            nc.vector.tensor_scalar_max(xr2[:], xn2[:], 1e-16)
            nc.vector.reciprocal(xr2[:], xr2[:])
            xr = pp.tile([P, NB], f32)
            nc.scalar.activation(xr[:], xr2[:], Act.Sqrt)
            xh = pp.tile([P, NB, D], bf16)
            xl = pp.tile([P, NB, D], bf16)
            for t in range(NB):
                nc.vector.tensor_scalar_mul(
                    x_s[:, t, :], x_s[:, t, :], xr[:, t : t + 1]
                )
                nc.vector.tensor_copy(xh[:, t, :], x_s[:, t, :])
                nc.vector.tensor_tensor(
                    out=xl[:, t, :], in0=x_s[:, t, :], in1=xh[:, t, :],
                    op=Alu.subtract,
                )
            xhT = pp.tile([P, G, B_LOCAL], bf16)
            xlT = pp.tile([P, G, B_LOCAL], bf16)
            for t in range(NB):
                nc.sync.dma_start_transpose(
                    xhT[:, :, t * P : (t + 1) * P], xh[:, t, :]
                )
                nc.sync.dma_start_transpose(
                    xlT[:, :, t * P : (t + 1) * P], xl[:, t, :]
                )

            # candidate arrays (persist across chunk loop)
            cand_v = [pp.tile([P, NCH * 8], f32, tag=f"cv{m}", name=f"cv{m}_{_rep}") for m in range(NB)]
            cand_i = [pp.tile([P, NCH * 8], f32, tag=f"ci{m}", name=f"ci{m}_{_rep}") for m in range(NB)]

            # ---------------- codebook streaming ----------------
            for j in range(NCH):
                c_s = sp.tile([P, NB, D], f32, tag="c_s")
                nc.sync.dma_start(
                    c_s[:],
                    cb.ap()[j * CHUNK : (j + 1) * CHUNK, :].rearrange(
                        "(t p) d -> p t d", p=P
                    ),
                )
                nrm2 = sp.tile([P, NB], f32, tag="nrm2")
                cjunk = sp.tile([P, D], bf16, tag="cjunk")
                for t in range(NB):
                    nc.scalar.activation(
                        cjunk[:], c_s[:, t, :], Act.Square,
                        accum_out=nrm2[:, t : t + 1],
                    )
                nc.vector.tensor_scalar_max(nrm2[:], nrm2[:], 1e-16)
                nc.vector.reciprocal(nrm2[:], nrm2[:])
                rcp = sp.tile([P, NB], f32, tag="rcp")
                nc.scalar.activation(rcp[:], nrm2[:], Act.Sqrt)

                ch = sp.tile([P, NB, D], bf16, tag="ch")
                cl = sp.tile([P, NB, D], bf16, tag="cl")
                for t in range(NB):
                    nc.vector.tensor_scalar_mul(
                        c_s[:, t, :], c_s[:, t, :], rcp[:, t : t + 1]
                    )
                    nc.vector.tensor_copy(ch[:, t, :], c_s[:, t, :])
                    nc.vector.tensor_tensor(
                        out=cl[:, t, :], in0=c_s[:, t, :], in1=ch[:, t, :],
                        op=Alu.subtract,
                    )
                chT = sp.tile([P, G, CHUNK], bf16, tag="chT")
                clT = sp.tile([P, G, CHUNK], bf16, tag="clT")
                for t in range(NB):
                    nc.sync.dma_start_transpose(
                        chT[:, :, t * P : (t + 1) * P], ch[:, t, :]
                    )
                    nc.sync.dma_start_transpose(
                        clT[:, :, t * P : (t + 1) * P], cl[:, t, :]
                    )

                for m in range(NB):
                    pm = ps.tile([P, CHUNK], f32, tag=f"pm{m}", space="PSUM")
                    for g in range(G):
                        nc.tensor.matmul(
                            pm[:], xhT[:, g, m * P : (m + 1) * P], chT[:, g, :],
                            start=(g == 0), stop=False,
                        )
                    for g in range(G):
                        nc.tensor.matmul(
                            pm[:], xhT[:, g, m * P : (m + 1) * P], clT[:, g, :],
                            start=False, stop=False,
                        )
                    for g in range(G):
                        nc.tensor.matmul(
                            pm[:], xlT[:, g, m * P : (m + 1) * P], chT[:, g, :],
                            start=False, stop=(g == G - 1),
                        )
                    sims = sp.tile([P, CHUNK], f32, tag=f"sims{m}")
                    nc.scalar.copy(sims[:], pm[:])
                    cv8 = cand_v[m][:, j * 8 : (j + 1) * 8]
                    nc.vector.max(out=cv8, in_=sims[:])
                    tmpi = sp.tile([P, 8], u32, tag=f"tmpi{m}")
                    nc.vector.max_index(out=tmpi[:], in_max=cv8, in_values=sims[:])
                    nc.vector.tensor_scalar(
                        out=cand_i[m][:, j * 8 : (j + 1) * 8],
                        in0=tmpi[:],
                        scalar1=float(j * CHUNK + 1),
                        scalar2=None,
                        op0=Alu.add,
                    )

            # ---------------- merge + select + gather ----------------
            NC = NCH * 8  # 256 candidates
            for m in range(NB):
                mr_a = sp.tile([P, NC], f32, tag="mr_a")
                mr_b = sp.tile([P, NC], f32, tag="mr_b")
                mx = sp.tile([P, 8], f32, tag="mx")
                src = cand_v[m]
                for r in range(4):
                    nc.vector.max(out=mx[:], in_=src[:])
                    if r < 3:
                        dst = mr_a if r % 2 == 0 else mr_b
                        nc.vector.match_replace(
                            out=dst[:], in_to_replace=mx[:], in_values=src[:],
                            imm_value=-1e30,
                        )
                        src = dst
                # threshold t32 = 32nd value = mx[:, 7] of round 3
                selm = sp.tile([P, NC], f32, tag="selm")
                nc.vector.tensor_scalar(
                    out=selm[:], in0=cand_v[m][:], scalar1=mx[:, 7:8],
                    scalar2=None, op0=Alu.is_ge,
                )
                seli = sp.tile([P, NC], f32, tag="seli")
                nc.vector.tensor_tensor(
                    out=seli[:], in0=selm[:], in1=cand_i[m][:], op=Alu.mult
                )
                win = sp.tile([P, K], f32, tag="win")
                srci = seli
                for r in range(4):
                    w8 = win[:, r * 8 : (r + 1) * 8]
                    nc.vector.max(out=w8, in_=srci[:])
                    if r < 3:
                        dsti = mr_a if r % 2 == 0 else mr_b
                        nc.vector.match_replace(
                            out=dsti[:], in_to_replace=w8, in_values=srci[:],
                            imm_value=0.0,
                        )
                        srci = dsti
                idx32 = sp.tile([P, K], i32, tag="idx32")
                nc.vector.tensor_scalar(
                    out=idx32[:], in0=win[:], scalar1=-1.0, scalar2=None,
                    op0=Alu.add,
                )
                xh_m = sp.tile([P, D], f32, tag="xh_m")
                for q in range(K):
                    nc.gpsimd.indirect_dma_start(
                        out=xh_m[:],
                        out_offset=None,
                        in_=cb[:, :],
                        in_offset=bass.IndirectOffsetOnAxis(
                            ap=idx32[:, q : q + 1], axis=0
                        ),
                        compute_op=(Alu.bypass if q == 0 else Alu.add),
                    )
                nc.sync.dma_start(xhat[m * P : (m + 1) * P, :], xh_m[:])

    _split_multi_waits(nc, mybir)
    return nc


_cached = {}


def _get_kernel(reps=1):
    if reps not in _cached:
        _cached[reps] = build_kernel(reps)
    return _cached[reps]


def kernel(x, codebook, k):
    from concourse.bass_utils import run_bass_kernel_spmd

    x = np.ascontiguousarray(np.asarray(x, dtype=np.float32))
    codebook = np.ascontiguousarray(np.asarray(codebook, dtype=np.float32))
    assert int(k) == K, f"kernel hardcodes k={K}, got {k}"
    assert x.shape == (N_CORES * B_LOCAL, D)
    assert codebook.shape == (NCB, D)

    nc = _get_kernel()
    in_maps = [
        {"x": x[i * B_LOCAL : (i + 1) * B_LOCAL], "codebook": codebook}
        for i in range(N_CORES)
    ]
    res = run_bass_kernel_spmd(nc, in_maps, list(range(N_CORES)))
    out = np.concatenate([r["x_hat"] for r in res.results], axis=0)
    return out.astype(np.float32)


if __name__ == "__main__":
    rng = np.random.default_rng(0)
    x = rng.standard_normal((N_CORES * B_LOCAL, D), dtype=np.float32)
    cbk = rng.standard_normal((NCB, D), dtype=np.float32)
    out = kernel(x, cbk, K)
    print("kernel ran, out shape", out.shape)
